# revision 1
# baseline (speedup 1.0000x reference)
"""Trainium2 Bass kernel for the EnergyBasedModel (equilibrium propagation)
negative-phase fixed-point iteration.

Strategy (pure data-parallel over batch, 8 cores):
 - batch 8192 -> 1024 rows/core, kept FEATURE-MAJOR on chip ([feat, batch]).
 - states S1 [1024,B], S2 [512,B], S3 [10,B] live in SBUF fp32 the whole run.
 - sig(x) @ W0 is loop-invariant (states[0] is clamped): computed once into
   F1 = DT*(sig(x)@W0 + b0), stored bf16, re-added into PSUM each step via an
   identity matmul.
 - DT folded into the weights; b1 folded into an augmented K=11 matmul row
   (ones row of G3aug); b2 via a K=1 matmul against a ones row-vector.
 - per-step per-tile update: PSUM A = DT*(ff+fb+b); DVE computes
   pneg=(sig-1)*sig (bf16), bneg=pneg*A, then S = 1.1*S + bneg via the fused
   scalar_tensor_tensor; layer-3 additionally subtracts C3 = DT*BETA*onehot;
   ScalarE recomputes sig(S) in bf16 for the next step's matmuls.
 - batch is processed as 2 interleaved sub-batches of 512 so the elementwise
   tail of one sub-batch hides under the other's matmuls.
"""

import os
import numpy as np
import ml_dtypes

BF16 = ml_dtypes.bfloat16

DT = 0.1
BETA = 0.1
N_STEPS = 20
NUM_CLASSES = 10

B_TOT = 8192
N_CORES = 8
B = B_TOT // N_CORES          # 1024 rows per core
NSB = 2                       # sub-batches
SB = B // NSB                 # 512: matmul free dim / psum bank
D1 = 1024                     # layer-1 width
D2 = 512                      # layer-2 width
D3 = NUM_CLASSES              # 10
K1 = D1 // 128                # 8  k-tiles of layer-1 features
K2 = D2 // 128                # 4  k-tiles of layer-2 features
SIG1 = 0.7310585786300049     # sigmoid(1.0)

_BUILT = None                 # cached (nc, meta)


def _register_sigprime_mul():
    """Author a fused custom-DVE op: out = (in0^2 - in0) * in1 * s0.

    With in0 = sigmoid(S) and in1 = A (the PSUM matmul accumulation) this is
    -sigmoid'(S) * A * s0 in a single 1x DVE pass, replacing the two-op
    (pneg, bneg) chain. s0 carries the per-step state rescale factor.
    Registered through the documented custom-DVE authoring mechanism
    (concourse/dve_ops.py) at runtime; the sha pin is computed on the fly.
    """
    from concourse import dve_ops
    from concourse.dve_spec import Spec, Src0, Src1, C0, sq, lower
    from concourse.dve_spec import _has_src1
    from concourse.dve_uop import DveOpSpec

    name = "SIGPRIME_MUL_ANT"
    for op in dve_ops.OPS:
        if op.name == name:
            return op

    spec = Spec(
        body=(sq(Src0) - Src0) * Src1 * C0,
        reference=lambda in0, in1, s0, s1, imm2: (
            (in0.astype(np.float32) ** 2 - in0.astype(np.float32)) * in1 * s0),
    )
    row = dve_ops._CUSTOM_DVE_ROW_BASE + len(dve_ops.OPS)
    assert row < 0x20
    dve_ops._SUB_OPCODE_FOR_NAME[name] = row
    shas = {}
    for ver in ("v3", "v4"):
        shas[ver] = DveOpSpec(
            name=name, opcode=row, uops=lower(spec, ver=ver),
            rd1_en=_has_src1(spec)).sha(ver)
    op = dve_ops.DveOp(name, spec, subdim=False, uops_sha=shas,
                       perf_en={"v3": True, "v4": True})
    dve_ops.OPS.append(op)
    dve_ops.CUSTOM_DVE_SPECS[name] = spec
    return op


def _build(n_steps=N_STEPS, pool_m2=2, pool_s3=False, ps_bufs=6, ps3_bufs=2,
           asserts=False, scr_bufs=4, order="n_outer", probe=None,
           hw_reps=1, bneg_bf16=False, act_stage=False):
    import concourse.bass as bass
    import concourse.mybir as mybir
    import concourse.tile as tile
    from concourse import bacc
    from concourse.masks import make_identity

    SIGP = _register_sigprime_mul()

    f32 = mybir.dt.float32
    bf16 = mybir.dt.bfloat16
    Alu = mybir.AluOpType
    Act = mybir.ActivationFunctionType

    nc = bacc.Bacc("TRN2", target_bir_lowering=False, debug=False,
                   enable_asserts=asserts, num_devices=N_CORES)

    # ---- DRAM I/O ----
    xT_d = nc.dram_tensor("xT", [D1, B], f32, kind="ExternalInput")
    w0_d = nc.dram_tensor("w0", [D1, D1], bf16, kind="ExternalInput")
    w1_d = nc.dram_tensor("w1", [D1, D2], bf16, kind="ExternalInput")
    w1t_d = nc.dram_tensor("w1t", [D2, D1], bf16, kind="ExternalInput")
    w2_d = nc.dram_tensor("w2", [D2, D3], bf16, kind="ExternalInput")
    w2ta_d = nc.dram_tensor("w2t_aug", [D3 + 1, D2], bf16, kind="ExternalInput")
    b2r_d = nc.dram_tensor("b2r", [1, D3], bf16, kind="ExternalInput")
    b0c_d = nc.dram_tensor("b0c", [D1, 1], f32, kind="ExternalInput")
    c3_d = nc.dram_tensor("c3", [D3, B], f32, kind="ExternalInput")
    s1_d = nc.dram_tensor("s1", [D1, B], f32, kind="ExternalOutput")
    s2_d = nc.dram_tensor("s2", [D2, B], f32, kind="ExternalOutput")
    s3_d = nc.dram_tensor("s3", [D3, B], f32, kind="ExternalOutput")

    with tile.TileContext(nc) as tc:
        with (
            tc.tile_pool(name="persist", bufs=1) as pp,
            tc.tile_pool(name="winit", bufs=1) as wip,
            tc.tile_pool(name="xin", bufs=2) as xp,
            tc.tile_pool(name="ps", bufs=ps_bufs, space="PSUM") as psp,
            tc.tile_pool(name="ps3", bufs=ps3_bufs, space="PSUM") as ps3p,
            tc.tile_pool(name="scr", bufs=scr_bufs) as scr,
        ):
            # ---- persistent weights ----
            W1 = [pp.tile([128, D2], bf16, tag=f"W1_{k}", name=f"W1_{k}") for k in range(K1)]
            W1T = [pp.tile([128, D1], bf16, tag=f"W1T_{k}", name=f"W1T_{k}") for k in range(K2)]
            W2 = [pp.tile([128, D3], bf16, tag=f"W2_{k}", name=f"W2_{k}") for k in range(K2)]
            W2TA = pp.tile([D3 + 1, D2], bf16, tag="W2TA", name="W2TA")
            B2R = pp.tile([1, D3], bf16, tag="B2R", name="B2R")
            ONES1 = pp.tile([1, SB], bf16, tag="ONES1", name="ONES1")
            IDENT = pp.tile([128, 128], bf16, tag="IDENT", name="IDENT")
            B0C = [pp.tile([128, 1], f32, tag=f"B0C_{m}", name=f"B0C_{m}") for m in range(K1)]
            for k in range(K1):
                nc.sync.dma_start(W1[k][:], w1_d[128 * k:128 * (k + 1), :])
            for k in range(K2):
                nc.sync.dma_start(W1T[k][:], w1t_d[128 * k:128 * (k + 1), :])
                nc.sync.dma_start(W2[k][:], w2_d[128 * k:128 * (k + 1), :])
            nc.sync.dma_start(W2TA[:], w2ta_d[:])
            nc.sync.dma_start(B2R[:], b2r_d[:])
            for m in range(K1):
                nc.sync.dma_start(B0C[m][:], b0c_d[128 * m:128 * (m + 1), :])
            nc.vector.memset(ONES1[:], 1.0)
            make_identity(nc, IDENT[:])

            # ---- persistent state ----
            S1 = [[pp.tile([128, SB], f32, tag=f"S1_{m}_{n}", name=f"S1_{m}_{n}") for n in range(NSB)]
                  for m in range(K1)]
            S2 = [[pp.tile([128, SB], f32, tag=f"S2_{m}_{n}", name=f"S2_{m}_{n}") for n in range(NSB)]
                  for m in range(K2)]
            S3 = [pp.tile([D3, SB], f32, tag=f"S3_{n}", name=f"S3_{n}") for n in range(NSB)]
            G1 = [[pp.tile([128, SB], bf16, tag=f"G1_{m}_{n}", name=f"G1_{m}_{n}") for n in range(NSB)]
                  for m in range(K1)]
            G2 = [[pp.tile([128, SB], bf16, tag=f"G2_{m}_{n}", name=f"G2_{m}_{n}") for n in range(NSB)]
                  for m in range(K2)]
            G3A = [pp.tile([D3 + 1, SB], bf16, tag=f"G3A_{n}", name=f"G3A_{n}") for n in range(NSB)]
            F1 = [[pp.tile([128, SB], bf16, tag=f"F1_{m}_{n}", name=f"F1_{m}_{n}") for n in range(NSB)]
                  for m in range(K1)]
            C3 = [pp.tile([D3, SB], f32, tag=f"C3_{n}", name=f"C3_{n}") for n in range(NSB)]
            for n in range(NSB):
                nc.sync.dma_start(C3[n][:], c3_d[:, SB * n:SB * (n + 1)])
                nc.vector.memset(S3[n][:], 1.0)
                nc.vector.memset(G3A[n][0:D3, :], SIG1)
                # ones row lives at partition 10: engines can't address a
                # base partition of 10, so fill it via SBUF->SBUF DMA
                nc.sync.dma_start(G3A[n][D3:D3 + 1, :], ONES1[:])
                for m in range(K1):
                    nc.vector.memset(S1[m][n][:], 1.0)
                    nc.vector.memset(G1[m][n][:], SIG1)
                for m in range(K2):
                    nc.vector.memset(S2[m][n][:], 1.0)
                    nc.vector.memset(G2[m][n][:], SIG1)

            # ---- init: G0 = sig(xT) (bf16), F1 = DT*(W0^T G0 + b0) ----
            W0 = [wip.tile([128, D1], bf16, tag=f"W0_{k}", name=f"W0_{k}") for k in range(K1)]
            G0 = [wip.tile([128, B], bf16, tag=f"G0_{k}", name=f"G0_{k}") for k in range(K1)]
            for k in range(K1):
                nc.sync.dma_start(W0[k][:], w0_d[128 * k:128 * (k + 1), :])
                xt = xp.tile([128, B], f32, tag="xt", name="xt")
                nc.sync.dma_start(xt[:], xT_d[128 * k:128 * (k + 1), :])
                nc.scalar.activation(G0[k][:], xt[:], Act.Sigmoid)
            for m in range(K1):
                for n in range(NSB):
                    ps = psp.tile([128, SB], f32, tag="ps", name="ps")
                    for k in range(K1):
                        nc.tensor.matmul(ps[:], W0[k][:, 128 * m:128 * (m + 1)],
                                         G0[k][:, SB * n:SB * (n + 1)],
                                         start=(k == 0), stop=(k == K1 - 1))
                    nc.scalar.activation(F1[m][n][:], ps[:], Act.Identity,
                                         bias=B0C[m][:], scale=DT)

            # ---- main loop (scaled states) ----
            # The chip stores Shat_t = S_t / g^t (g = 1.1 for layers 1/2,
            # g3 = 1.11 for layer 3), so the update S <- g*S - sig'*A becomes
            # a pure add: Shat += (-sig'*A) * g^-(t+1). The g^-(t+1) rides the
            # custom op's s0; sigmoid reads true S via ACT's free input scale.
            g12 = 1.0 + DT
            g3 = 1.0 + DT + DT * BETA
            def emit_l1(t, n, m):
                sc12 = g12 ** -(t + 1)
                sg12 = g12 ** (t + 1)
                ps = psp.tile([128, SB], f32, tag="ps", name="ps")
                for k in range(K2):
                    nc.tensor.matmul(ps[:],
                                     W1T[k][:, 128 * m:128 * (m + 1)],
                                     G2[k][n][:],
                                     start=(k == 0), stop=False)
                nc.tensor.matmul(ps[:], IDENT[:], F1[m][n][:],
                                 start=False, stop=True)
                if probe == "mm_only":
                    drain = scr.tile([128, SB], bf16, tag="drain", name="drain")
                    nc.scalar.activation(drain[:], ps[:], Act.Identity)
                    return
                bneg = scr.tile([128, SB], bf16 if bneg_bf16 else f32,
                                tag="bneg", name="bneg")
                if act_stage:
                    abf = scr.tile([128, SB], bf16, tag="abf", name="abf")
                    nc.scalar.activation(abf[:], ps[:], Act.Identity)
                    nc.vector._custom_dve(SIGP, out=bneg[:],
                                          in0=G1[m][n][:], in1=abf[:],
                                          s0=sc12)
                else:
                    nc.vector._custom_dve(SIGP, out=bneg[:],
                                          in0=G1[m][n][:], in1=ps[:], s0=sc12)
                nc.gpsimd.tensor_add(S1[m][n][:], S1[m][n][:], bneg[:])
                nc.scalar.activation(G1[m][n][:], S1[m][n][:],
                                     Act.Sigmoid, scale=sg12)

            def emit_l2(t, n, m):
                sc12 = g12 ** -(t + 1)
                sg12 = g12 ** (t + 1)
                ps = psp.tile([128, SB], f32, tag="ps", name="ps")
                for k in range(K1):
                    nc.tensor.matmul(ps[:],
                                     W1[k][:, 128 * m:128 * (m + 1)],
                                     G1[k][n][:],
                                     start=(k == 0), stop=False)
                nc.tensor.matmul(ps[:],
                                 W2TA[:, 128 * m:128 * (m + 1)],
                                 G3A[n][:], start=False, stop=True)
                if probe == "mm_only":
                    drain = scr.tile([128, SB], bf16, tag="drain", name="drain")
                    nc.scalar.activation(drain[:], ps[:], Act.Identity)
                    return
                bneg = scr.tile([128, SB], bf16 if bneg_bf16 else f32,
                                tag="bneg", name="bneg")
                if act_stage:
                    abf = scr.tile([128, SB], bf16, tag="abf", name="abf")
                    nc.scalar.activation(abf[:], ps[:], Act.Identity)
                    nc.vector._custom_dve(SIGP, out=bneg[:],
                                          in0=G2[m][n][:], in1=abf[:],
                                          s0=sc12)
                else:
                    nc.vector._custom_dve(SIGP, out=bneg[:],
                                          in0=G2[m][n][:], in1=ps[:], s0=sc12)
                eng = nc.gpsimd if m < pool_m2 else nc.vector
                eng.tensor_add(S2[m][n][:], S2[m][n][:], bneg[:])
                nc.scalar.activation(G2[m][n][:], S2[m][n][:],
                                     Act.Sigmoid, scale=sg12)

            def emit_l3(t, n):
                sc3 = g3 ** -(t + 1)
                sg3 = g3 ** (t + 1)
                ps3 = ps3p.tile([D3, SB], f32, tag="ps3", name="ps3")
                for k in range(K2):
                    nc.tensor.matmul(ps3[:], W2[k][:], G2[k][n][:],
                                     start=(k == 0), stop=False)
                nc.tensor.matmul(ps3[:], B2R[:], ONES1[:],
                                 start=False, stop=True)
                if probe == "mm_only":
                    drain3 = scr.tile([D3, SB], bf16, tag="drain3", name="drain3")
                    nc.scalar.activation(drain3[:], ps3[:], Act.Identity)
                    return
                bneg3 = scr.tile([D3, SB], bf16 if bneg_bf16 else f32,
                                 tag="bneg3", name="bneg3")
                nc.vector._custom_dve(SIGP, out=bneg3[:],
                                      in0=G3A[n][0:D3, :], in1=ps3[:], s0=sc3)
                eng3 = nc.gpsimd if pool_s3 else nc.vector
                eng3.tensor_add(S3[n][:], S3[n][:], bneg3[:])
                # Shat3 -= C3 * g3^-(t+1):  (C3 * -sc3) + S3
                nc.vector.scalar_tensor_tensor(
                    S3[n][:], C3[n][:], -sc3, S3[n][:],
                    op0=Alu.mult, op1=Alu.add)
                nc.scalar.activation(G3A[n][0:D3, :], S3[n][:],
                                     Act.Sigmoid, scale=sg3)

            import contextlib
            loop_cm = (tc.For_i(0, hw_reps, 1) if hw_reps > 1
                       else contextlib.nullcontext())
            with loop_cm:
              for t in range(n_steps):
                if order == "n_outer":
                    for n in range(NSB):
                        for m in range(K1):
                            emit_l1(t, n, m)
                        for m in range(K2):
                            emit_l2(t, n, m)
                        emit_l3(t, n)
                elif order == "layer_outer":
                    for n in range(NSB):
                        for m in range(K1):
                            emit_l1(t, n, m)
                    for n in range(NSB):
                        for m in range(K2):
                            emit_l2(t, n, m)
                    for n in range(NSB):
                        emit_l3(t, n)
                else:  # "m_interleave"
                    for m in range(K1):
                        for n in range(NSB):
                            emit_l1(t, n, m)
                    for m in range(K2):
                        for n in range(NSB):
                            emit_l2(t, n, m)
                    for n in range(NSB):
                        emit_l3(t, n)

            # ---- rescale back to true S and store ----
            fin12 = g12 ** n_steps
            fin3 = g3 ** n_steps
            for m in range(K1):
                for n in range(NSB):
                    nc.scalar.activation(S1[m][n][:], S1[m][n][:], Act.Copy,
                                         scale=fin12)
                    nc.sync.dma_start(
                        s1_d[128 * m:128 * (m + 1), SB * n:SB * (n + 1)],
                        S1[m][n][:])
            for m in range(K2):
                for n in range(NSB):
                    nc.scalar.activation(S2[m][n][:], S2[m][n][:], Act.Copy,
                                         scale=fin12)
                    nc.sync.dma_start(
                        s2_d[128 * m:128 * (m + 1), SB * n:SB * (n + 1)],
                        S2[m][n][:])
            for n in range(NSB):
                nc.scalar.activation(S3[n][:], S3[n][:], Act.Copy, scale=fin3)
                nc.sync.dma_start(s3_d[:, SB * n:SB * (n + 1)], S3[n][:])

    nc.compile()
    return nc


def get_built(n_steps=N_STEPS):
    global _BUILT
    if _BUILT is None or _BUILT[0] != n_steps:
        _BUILT = (n_steps, _build(n_steps))
    return _BUILT[1]


def _prep_core_inputs(x, target, W0, W1, W2, b0, b1, b2):
    """Host-side preprocessing -> list of per-core input dicts."""
    x = np.asarray(x, np.float32)
    target = np.asarray(target)
    W0 = np.asarray(W0, np.float32)
    W1 = np.asarray(W1, np.float32)
    W2 = np.asarray(W2, np.float32)
    b0 = np.asarray(b0, np.float32)
    b1 = np.asarray(b1, np.float32)
    b2 = np.asarray(b2, np.float32)

    w0 = W0.astype(BF16)                              # unscaled; DT applied in F1 pass
    w1 = (DT * W1).astype(BF16)
    w1t = np.ascontiguousarray((DT * W1).T).astype(BF16)
    w2 = (DT * W2).astype(BF16)
    w2ta = np.concatenate([(DT * W2).T, (DT * b1)[None, :]], axis=0).astype(BF16)
    b2r = (DT * b2)[None, :].astype(BF16)
    b0c = (DT * b0)[:, None].astype(np.float32)

    onehot = np.zeros((B_TOT, NUM_CLASSES), np.float32)
    onehot[np.arange(B_TOT), target.astype(np.int64)] = 1.0

    in_maps = []
    for c in range(N_CORES):
        sl = slice(c * B, (c + 1) * B)
        xT = np.ascontiguousarray(x[sl].T)            # [1024, B]
        c3 = np.ascontiguousarray((DT * BETA) * onehot[sl].T)  # [10, B]
        in_maps.append({
            "xT": xT, "w0": w0, "w1": w1, "w1t": w1t, "w2": w2,
            "w2t_aug": w2ta, "b2r": b2r, "b0c": b0c, "c3": c3,
        })
    return in_maps


_RUNNER = None


def _get_runner(nc):
    """Build the sharded PJRT callable once and reuse it across kernel()
    calls (run_bass_kernel_spmd re-jits + re-loads the NEFF every call)."""
    global _RUNNER
    if _RUNNER is not None:
        return _RUNNER
    import jax
    from jax.sharding import Mesh, PartitionSpec
    from jax.experimental.shard_map import shard_map
    import concourse.mybir as mybir
    from concourse.bass2jax import (_bass_exec_p, install_neuronx_cc_hook,
                                    partition_id_tensor)

    install_neuronx_cc_hook()
    partition_name = (nc.partition_id_tensor.name
                      if nc.partition_id_tensor else None)
    in_names, out_names, out_avals, zero_outs = [], [], [], []
    for alloc in nc.m.functions[0].allocations:
        if not isinstance(alloc, mybir.MemoryLocationSet):
            continue
        name = alloc.memorylocations[0].name
        if alloc.kind == "ExternalInput":
            if name != partition_name:
                in_names.append(name)
        elif alloc.kind == "ExternalOutput":
            shape = tuple(alloc.tensor_shape)
            dtype = mybir.dt.np(alloc.dtype)
            out_names.append(name)
            out_avals.append(jax.core.ShapedArray(shape, dtype))
            zero_outs.append(np.zeros(shape, dtype))
    n_params, n_outs = len(in_names), len(out_avals)
    all_names = in_names + out_names
    if partition_name is not None:
        all_names.append(partition_name)

    def _body(*args):
        operands = list(args)
        if partition_name is not None:
            operands.append(partition_id_tensor())
        return tuple(_bass_exec_p.bind(
            *operands, out_avals=tuple(out_avals), in_names=tuple(all_names),
            out_names=tuple(out_names), lowering_input_output_aliases=(),
            sim_require_finite=True, sim_require_nnan=True, nc=nc))

    devices = jax.devices()[:N_CORES]
    mesh = Mesh(np.asarray(devices), ("core",))
    in_specs = (PartitionSpec("core"),) * (n_params + n_outs)
    out_specs = (PartitionSpec("core"),) * n_outs
    fn = jax.jit(shard_map(_body, mesh=mesh, in_specs=in_specs,
                           out_specs=out_specs, check_rep=False),
                 donate_argnums=tuple(range(n_params, n_params + n_outs)),
                 keep_unused=True)

    def run(in_maps):
        per_core = [[np.asarray(m[name]) for name in in_names]
                    for m in in_maps]
        concat_in = [np.concatenate([per_core[c][i] for c in range(N_CORES)],
                                    axis=0) for i in range(n_params)]
        zeros = [np.zeros((N_CORES * z.shape[0], *z.shape[1:]), z.dtype)
                 for z in zero_outs]
        out = jax.block_until_ready(fn(*concat_in, *zeros))
        return [
            {name: np.asarray(out[i]).reshape(N_CORES, *out_avals[i].shape)[c]
             for i, name in enumerate(out_names)}
            for c in range(N_CORES)
        ]

    _RUNNER = run
    return run


def kernel(x, target, W0, W1, W2, b0, b1, b2):
    n_steps = int(os.environ.get("EBM_N_STEPS", N_STEPS))
    nc = get_built(n_steps)
    in_maps = _prep_core_inputs(x, target, W0, W1, W2, b0, b1, b2)
    try:
        results = _get_runner(nc)(in_maps)
    except Exception:
        from concourse import bass_utils
        results = bass_utils.run_bass_kernel_spmd(
            nc, in_maps, list(range(N_CORES))).results

    x = np.asarray(x, np.float32)
    outs = []
    for c in range(N_CORES):
        r = results[c]
        sl = slice(c * B, (c + 1) * B)
        outs.append(np.concatenate(
            [x[sl], r["s1"].T, r["s2"].T, r["s3"].T], axis=1))
    return np.concatenate(outs, axis=0).astype(np.float32)

